# revision 21
# baseline (speedup 1.0000x reference)
"""Trainium2 Bass kernel for the 8-step attentive LSTM ("read-process"
FLayer): B=32, T=128, E=1024, N*k=320 support vectors, K_STEPS=8.

Sharding: data-parallel over B across 8 NeuronCores (4 episodes/core), LSTM
weights replicated, the sequential K loop stays local per core. No collectives.

The dominant matmuls run in fp8-e4m3 DoubleRow mode (two 128-deep contraction
chunks per instruction at 0.5 cycles/row - 4x the bf16 rate) with a precision
scheme validated against the f32 reference in numpy (rel_err 8.3e-3, budget
2e-2; the bf16 baseline measured 1.3e-3 at 582us):

  * u-form recurrence: u(t) = o*tanh(c) + r(t-1) (= h_in - x), so the
    per-step matmul is gates = W_hh.u + Z with Z = (W_ih+W_hh).x + b
    precomputed once; the x part of the moving operand is never
    re-quantized (measurably better than quantizing h = u + x).
  * weights/Z pre-scaled by LAM=16 (escapes fp8 subnormals; i,f,o rows also
    fold the sigmoid-via-tanh 0.5), descaled for free by the ACT tanh input
    scale.  sigmoid(x) = 0.5 tanh(x/2)+0.5 is fused into scalar_tensor_tensor
    ops on a doubled cell state C = 2c, so the cell costs 2 batched ACT tanh
    + 1 tanh(C/2) + 4 fused DVE/Pool ops per E-chunk, and every ACT func
    (tanh/exp/identity) lives in the single "exp_and_others" table.
  * 2-limb fp8 (value + unscaled residual; the residual lands in fp8's
    subnormal range where absolute error beats bf16): W_hh on odd steps, Z
    always (added inside the matmul accumulation by an fp8 [I,I] DoubleRow
    pair straight from the resident 2-limb Z), dots_x likewise.
  * chaos containment: step-0 gates and step-1/2 logits in exact f32 (f32r
    matmuls run at bf16 speed), r(t) through a fp16 S copy at every step
    (fp16 r-matmul = 1.0 cy/row, precision ~0.05%); from step 3 the softmax
    is saturated (top-gap ~800) and logits run 1-limb fp8.
  * the final h = 0.5*A2 + x add runs on the host (x exact there); the
    device returns A2 = (t_o+1)*tanh(c) in fp16.

Scheduling: gate PSUM is double-buffered in half-chunks (i,f | g,o) so PE
never waits on the ACT drain; per E-chunk the r-matmuls of all 4 episodes
land in one PSUM bank so u(t+1) assembly is a single fused stt; attention
r-matmuls are interleaved into the gates loop (after the chunk's gates) to
keep the in-order PE queue deadlock-free with 2 PSUM bufs.
"""

import os
import sys

for _p in ("/opt/trn_rl_repo", "/root/.axon_site/_ro/trn_rl_repo"):
    if os.path.isdir(_p) and _p not in sys.path:
        sys.path.insert(0, _p)

import numpy as np
import ml_dtypes

import concourse.bass as bass
import concourse.mybir as mybir
import concourse.tile as tile
from concourse import bacc
from concourse.bass_utils import run_bass_kernel_spmd

# Problem shape (hardcoded per contract)
B, T, E, NK, KS = 32, 128, 1024, 320, 8
CORES = 8
EPC = B // CORES          # episodes per core = 4
R = EPC * T               # rows per core = 512
G = 4 * E                 # gate rows = 4096
EC = E // 128             # E chunks = 8
GT = G // 128             # gate tiles = 32
KP = EC // 2              # contraction pairs = 4
NKP = 384                 # padded support count (3 chunks of 128)
NC = NKP // 128           # nk chunks = 3
LAM = 16.0                # weight prescale (fp8 subnormal escape)
WLO_STEPS = (1, 3, 5, 7)  # steps that add the W_hh low limb

F32 = mybir.dt.float32
F32R = mybir.dt.float32r
BF16 = mybir.dt.bfloat16
FP16 = mybir.dt.float16
FP8 = mybir.dt.float8e4
AF = mybir.ActivationFunctionType
ALU = mybir.AluOpType
AX = mybir.AxisListType
DR = mybir.MatmulPerfMode.DoubleRow
F8NP = ml_dtypes.float8_e4m3
F16NP = np.float16

_STATE = {}


def _build(n_steps=KS):
    nc = bacc.Bacc("TRN2", target_bir_lowering=False, debug=False,
                   enable_asserts=True)

    # ---- DRAM parameters (per-core shards; host preps layouts/dtypes) ----
    wih = nc.dram_tensor("wih", [GT, 128, EC, 128], F32R, kind="ExternalInput").ap()
    w8h = nc.dram_tensor("w8h", [EC, 128, KP, 2, 512], FP8, kind="ExternalInput").ap()
    w8l = nc.dram_tensor("w8l", [EC, 128, KP, 2, 512], FP8, kind="ExternalInput").ap()
    x8 = nc.dram_tensor("x8", [2, E, R], FP8, kind="ExternalInput").ap()
    xtf = nc.dram_tensor("xtf", [E, R], F32R, kind="ExternalInput").ap()
    bt_d = nc.dram_tensor("bias_t", [128, GT], F32, kind="ExternalInput").ap()
    bz_d = nc.dram_tensor("bias_z", [128, GT], F32, kind="ExternalInput").ap()
    stf = nc.dram_tensor("stf", [EPC, E, NKP], F32R, kind="ExternalInput").ap()
    s8d = nc.dram_tensor("s8d", [EPC, E, NKP], FP8, kind="ExternalInput").ap()
    s16 = nc.dram_tensor("s16", [EPC, NC, 128, E], FP16, kind="ExternalInput").ap()
    r0d = nc.dram_tensor("r0", [EC, 128, EPC], F32, kind="ExternalInput").ap()
    i8d = nc.dram_tensor("i8", [128, 2, 128], FP8, kind="ExternalInput").ap()
    i16d = nc.dram_tensor("i16", [128, 128], FP16, kind="ExternalInput").ap()
    a2o = nc.dram_tensor("a2o", [128, EC, R], FP16, kind="ExternalOutput").ap()
    dbg = {}
    if os.environ.get("K_DEBUG"):
        for t in range(KS):
            dbg[f"u{t}"] = nc.dram_tensor(f"dbg_u{t}", [128, EC, R], FP8,
                                          kind="ExternalOutput").ap()
            dbg[f"a{t}"] = nc.dram_tensor(f"dbg_a{t}", [128, EC, R], FP16,
                                          kind="ExternalOutput").ap()

    with tile.TileContext(nc) as tc:
        with (
            tc.tile_pool(name="res", bufs=1) as res,
            tc.tile_pool(name="pxw", bufs=4) as pxw,
            tc.tile_pool(name="pw8l", bufs=2) as pw8l,
            tc.tile_pool(name="pgw", bufs=2) as pgw,
            tc.tile_pool(name="ptmp", bufs=3) as ptmp,
            tc.tile_pool(name="pth", bufs=2) as pth,
            tc.tile_pool(name="pbig", bufs=2) as pbig,
            tc.tile_pool(name="pat", bufs=2) as pat,
            tc.tile_pool(name="pstat", bufs=8) as pstat,
            tc.tile_pool(name="ppg", bufs=2, space="PSUM") as ppg_pool,
            tc.tile_pool(name="pp1", bufs=2, space="PSUM") as pp1,
            tc.tile_pool(name="pp2", bufs=2, space="PSUM") as pp2,
        ):
            # ---- resident tiles ----
            w8h_sb = res.tile([128, EC, KP, 2, 512], FP8, tag="w8h")
            z8_sb = res.tile([128, EC, 2, 4, R], FP8, tag="z8")
            x8h_sb = res.tile([128, EC, R], FP8, tag="x8h")
            x8l_sb = res.tile([128, EC, R], FP8, tag="x8l")
            xtf_sb = res.tile([128, EC, R], F32R, tag="xtf")
            s8d_sb = res.tile([128, EPC, EC, NKP], FP8, tag="s8d")
            s16_sb = res.tile([128, EPC, NC, E], FP16, tag="s16")
            dx8_sb = res.tile([128, EPC, 2, NKP], FP8, tag="dx8")
            aT_sb = res.tile([128, EPC, NC, 128], FP16, tag="aT")
            r1_sb = res.tile([128, EC, R], FP16, tag="r1")
            u8a = res.tile([128, EC, R], FP8, tag="u8a")
            u8b = res.tile([128, EC, R], FP8, tag="u8b")
            cC = res.tile([128, EC, R], FP16, tag="cC")     # doubled cell 2c
            A2 = res.tile([128, EC, R], FP16, tag="A2")     # (t_o+1)*tanh(c)
            bt_sb = res.tile([128, GT], F32, tag="bt")
            bz_sb = res.tile([128, GT], F32, tag="bz")
            r0_sb = res.tile([128, EC, EPC], F32, tag="r0")
            i8_sb = res.tile([128, 2, 128], FP8, tag="i8")
            i16_sb = res.tile([128, 128], FP16, tag="i16")

            # resident loads: what the first zbuild/Xp tiles need comes first
            # (x8 limbs + w8h block 0 on ACT queue, xtf on SP); the rest
            # trickles in behind.
            nc.scalar.dma_start(out=x8h_sb[:],
                                in_=x8[0].rearrange("(k p) c -> p k c", p=128))
            nc.scalar.dma_start(out=x8l_sb[:],
                                in_=x8[1].rearrange("(k p) c -> p k c", p=128))
            nc.sync.dma_start(out=xtf_sb[:],
                              in_=xtf.rearrange("(k p) c -> p k c", p=128))
            for e in range(EC):
                nc.scalar.dma_start(out=w8h_sb[:, e, :, :, :], in_=w8h[e])
            nc.scalar.dma_start(out=bt_sb[:], in_=bt_d[:])
            nc.scalar.dma_start(out=bz_sb[:], in_=bz_d[:])
            nc.scalar.dma_start(out=r0_sb[:], in_=r0d.rearrange("k p e -> p k e"))

            def u8pair(u8t, kp, cols=slice(None)):
                return u8t[:, 2 * kp:2 * kp + 2, cols]

            # ================= phase B =================
            # Z = (W_ih + W_hh).x + b, prescaled by LAM*rs.  W_ih part exact
            # f32r (doubles as the chaos-sensitive step-0 gates); W_hh part
            # rebuilt from the resident fp8 limbs (err ~0.2%, folded into the
            # 2-limb Z storage).
            for e in range(EC):
                ms = (e, 8 + e, 16 + e, 24 + e)     # i, f, g, o gate tiles
                w8lt = pw8l.tile([128, KP, 2, 512], FP8, tag="w8l")
                (nc.sync if e % 2 == 0 else nc.scalar).dma_start(
                    out=w8lt[:], in_=w8l[e])
                tg0 = pgw.tile([128, 4, R], FP16, tag="gw")
                for half in range(2):
                    ppg = ppg_pool.tile([128, 2, R], F32, tag="pg")
                    for hi in range(2):
                        gi = 2 * half + hi
                        m = ms[gi]
                        # fp8 W_hh.x limb products
                        zps = pp1.tile([128, R], F32, tag="p1")
                        first = True
                        for pi, (hi_limb, xt) in enumerate(
                                ((True, x8h_sb), (False, x8h_sb),
                                 (True, x8l_sb))):
                            wt = w8h_sb[:, e] if hi_limb else w8lt
                            for kp in range(KP):
                                nc.tensor.matmul(
                                    zps[:],
                                    wt[:, kp, :, gi * 128:(gi + 1) * 128],
                                    u8pair(xt, kp), start=first,
                                    stop=(pi == 2 and kp == KP - 1),
                                    perf_mode=DR)
                                first = False
                        zbt = ptmp.tile([128, R], FP16, tag="tmp", name="zb")
                        nc.scalar.activation(zbt[:], zps[:], AF.Identity)
                        # exact f32r W_ih.x into the gate slot; the weight
                        # stream rides both HWDGE queues in half tiles
                        hf = EC // 2
                        xwa = pxw.tile([128, hf, 128], F32R, tag="xw",
                                       name="xwa")
                        xwb = pxw.tile([128, hf, 128], F32R, tag="xw",
                                       name="xwb")
                        nc.sync.dma_start(out=xwa[:], in_=wih[m][:, :hf, :])
                        nc.scalar.dma_start(out=xwb[:], in_=wih[m][:, hf:, :])
                        for k in range(EC):
                            lhs = xwa[:, k, :] if k < hf else xwb[:, k - hf, :]
                            nc.tensor.matmul(ppg[:, hi, :], lhs,
                                             xtf_sb[:, k, :], start=(k == 0),
                                             stop=(k == EC - 1))
                        # step-0 gate tanh + Z limb assembly
                        nc.scalar.activation(tg0[:, gi, :], ppg[:, hi, :],
                                             AF.Tanh, bias=bt_sb[:, m:m + 1],
                                             scale=1.0 / LAM)
                        zf = ptmp.tile([128, R], FP16, tag="tmp", name="zf")
                        nc.vector.scalar_tensor_tensor(
                            zf[:], ppg[:, hi, :], bz_sb[:, m:m + 1], zbt[:],
                            op0=ALU.add, op1=ALU.add)
                        nc.gpsimd.tensor_copy(z8_sb[:, e, 0, gi, :], zf[:])
                        nc.vector.tensor_sub(z8_sb[:, e, 1, gi, :], zf[:],
                                             z8_sb[:, e, 0, gi, :])
                # step-0 cell (h=c=r=0): C0 = (t_i+1)*t_g; A2 = (t_o+1)tanh(C0/2)
                nc.gpsimd.scalar_tensor_tensor(cC[:, e, :], tg0[:, 0, :], 1.0,
                                               tg0[:, 2, :], op0=ALU.add,
                                               op1=ALU.mult)
                thc = pth.tile([128, R], FP16, tag="th")
                nc.scalar.activation(thc[:], cC[:, e, :], AF.Tanh, scale=0.5)
                nc.gpsimd.scalar_tensor_tensor(A2[:, e, :], tg0[:, 3, :], 1.0,
                                               thc[:], op0=ALU.add, op1=ALU.mult)
                # u(1) = 0.5*A2 + r0 (r0 = per-episode support mean, exact)
                for ep in range(EPC):
                    cs = slice(ep * 128, (ep + 1) * 128)
                    nc.gpsimd.tensor_scalar(u8a[:, e, cs], A2[:, e, cs], 0.5,
                                            r0_sb[:, e, ep:ep + 1],
                                            op0=ALU.mult, op1=ALU.add)
            # loads first needed from step 1 on, deferred so the phase-B
            # weight stream owns the queues
            nc.scalar.dma_start(out=i8_sb[:], in_=i8d[:])
            nc.scalar.dma_start(out=i16_sb[:], in_=i16d[:])
            nc.scalar.dma_start(out=s16_sb[:],
                                in_=s16.rearrange("ep c p e -> p ep c e"))
            nc.scalar.dma_start(out=s8d_sb[:],
                                in_=s8d.rearrange("ep (k p) c -> p ep k c", p=128))
            if dbg:
                nc.sync.dma_start(out=dbg["u0"], in_=u8a[:])
                nc.sync.dma_start(out=dbg["a0"], in_=A2[:])

            # ================= steps 1..7 =================
            for t in range(1, n_steps):
                last = (t == n_steps - 1)
                wlo = t in WLO_STEPS
                u8cur, u8nxt = (u8a, u8b) if t % 2 == 1 else (u8b, u8a)

                # ---- attention front: logits/softmax/transposes per ep ----
                if not last:
                    for ep in range(EPC):
                        cs = slice(ep * 128, (ep + 1) * 128)
                        psd = pp1.tile([128, NKP], F32, tag="p1", name="psd")
                        if t <= 2:
                            # exact f32r logits (steps 1-2 are chaos-sensitive)
                            half = EC // 2
                            sview = stf[ep].rearrange("(k p) c -> p k c", p=128)
                            spa = pbig.tile([128, half, NKP], F32R, tag="big",
                                            name="spa")
                            spb = pbig.tile([128, half, NKP], F32R, tag="big",
                                            name="spb")
                            nc.sync.dma_start(out=spa[:], in_=sview[:, :half, :])
                            nc.scalar.dma_start(out=spb[:], in_=sview[:, half:, :])
                            # q tiles overlay the x8 limb tiles (dead after
                            # phase B): fp8 [128,EC,512] viewed as f32
                            # [128,EC,128], ping-ponged across episodes.
                            qt = (x8h_sb if ep % 2 == 0 else
                                  x8l_sb)[:].bitcast(F32)
                            if t == 1:
                                # q1 = 0.5*A2(0) + x + r0, exact
                                nc.gpsimd.scalar_tensor_tensor(
                                    qt[:], A2[:, :, cs], 0.5,
                                    xtf_sb[:, :, cs].bitcast(F32),
                                    op0=ALU.mult, op1=ALU.add)
                                for k in range(EC):
                                    nc.gpsimd.tensor_scalar_add(
                                        qt[:, k, :], qt[:, k, :],
                                        r0_sb[:, k, ep:ep + 1])
                            else:
                                # q2 = 0.5*A2(1) + r1 + x (r1 fp16-exact)
                                nc.gpsimd.scalar_tensor_tensor(
                                    qt[:], A2[:, :, cs], 0.5, r1_sb[:, :, cs],
                                    op0=ALU.mult, op1=ALU.add)
                                nc.gpsimd.tensor_add(
                                    qt[:], qt[:], xtf_sb[:, :, cs].bitcast(F32))
                            for k in range(EC):
                                rhs = (spa[:, k, :] if k < half
                                       else spb[:, k - half, :])
                                nc.tensor.matmul(psd[:],
                                                 qt[:, k, :].bitcast(F32R), rhs,
                                                 start=(k == 0),
                                                 stop=(k == EC - 1))
                            if t == 2:
                                # dots_x = x.S exact -> fp8 limbs for steps 3+
                                dxp = pp1.tile([128, NKP], F32, tag="p1",
                                               name="dxp")
                                for k in range(EC):
                                    rhs = (spa[:, k, :] if k < half
                                           else spb[:, k - half, :])
                                    nc.tensor.matmul(dxp[:], xtf_sb[:, k, cs],
                                                     rhs, start=(k == 0),
                                                     stop=(k == EC - 1))
                                nc.scalar.activation(dx8_sb[:, ep, 0, :],
                                                     dxp[:], AF.Identity)
                                nc.vector.tensor_sub(dx8_sb[:, ep, 1, :],
                                                     dxp[:],
                                                     dx8_sb[:, ep, 0, :])
                        else:
                            # fp8 logits: dots_x pair + u8.S8h
                            nc.tensor.matmul(psd[:], i8_sb[:],
                                             dx8_sb[:, ep, :, :], start=True,
                                             stop=False, perf_mode=DR)
                            for kp in range(KP):
                                nc.tensor.matmul(
                                    psd[:], u8pair(u8cur, kp, cs),
                                    s8d_sb[:, ep, 2 * kp:2 * kp + 2, :],
                                    start=False, stop=(kp == KP - 1),
                                    perf_mode=DR)
                        # softmax over NKP (padding exps to ~0 automatically)
                        nmax = pstat.tile([128, 1], F32, tag="st")
                        nc.vector.tensor_reduce(nmax[:], psd[:], axis=AX.X,
                                                op=ALU.max, negate=True)
                        ate = pat.tile([128, NKP], FP16, tag="ate")
                        sm = pstat.tile([128, 1], F32, tag="st")
                        nc.scalar.activation(ate[:], psd[:], AF.Exp,
                                             bias=nmax[:, :], accum_out=sm[:, :])
                        rec = pstat.tile([128, 1], F32, tag="st")
                        nc.vector.reciprocal(rec[:], sm[:])
                        at16 = pat.tile([128, NKP], FP16, tag="at16")
                        nc.gpsimd.tensor_scalar_mul(at16[:], ate[:], rec[:, :])
                        for c in range(NC):
                            pt = pp2.tile([128, 128], FP16, tag="p2", name="pt")
                            nc.tensor.transpose(
                                pt[:], at16[:, c * 128:(c + 1) * 128],
                                i16_sb[:, :])
                            nc.vector.tensor_copy(aT_sb[:, ep, c, :], pt[:])

                # ---- gates + cell update + interleaved r-matmuls ----
                for e in range(EC):
                    zt = z8_sb[:, e]
                    if wlo:
                        w8lt = pw8l.tile([128, KP, 2, 512], FP8, tag="w8l")
                        (nc.sync if e % 2 == 0 else nc.scalar).dma_start(
                            out=w8lt[:], in_=w8l[e])
                    tg = pgw.tile([128, 4, R], FP16, tag="gw")
                    for half in range(2):
                        ppg = ppg_pool.tile([128, 2, R], F32, tag="pg")
                        for hi in range(2):
                            gi = 2 * half + hi
                            slot = ppg[:, hi, :]
                            first = True
                            if wlo:
                                for kp in range(KP):
                                    nc.tensor.matmul(
                                        slot,
                                        w8lt[:, kp, :, gi * 128:(gi + 1) * 128],
                                        u8pair(u8cur, kp), start=first,
                                        stop=False, perf_mode=DR)
                                    first = False
                            for kp in range(KP):
                                nc.tensor.matmul(
                                    slot,
                                    w8h_sb[:, e, kp, :, gi * 128:(gi + 1) * 128],
                                    u8pair(u8cur, kp), start=first, stop=False,
                                    perf_mode=DR)
                                first = False
                            nc.tensor.matmul(slot, i8_sb[:], zt[:, :, gi, :],
                                             start=False, stop=True,
                                             perf_mode=DR)
                        nc.scalar.activation(tg[:, 2 * half:2 * half + 2, :],
                                             ppg[:], AF.Tanh, scale=1.0 / LAM)
                    # r-matmuls for this E-chunk: all 4 episodes into one
                    # PSUM bank (fp16 S, one accumulation group)
                    psr4 = None
                    if not last:
                        es = slice(e * 128, (e + 1) * 128)
                        psr4 = pp2.tile([128, EPC, 128], F32, tag="p2",
                                        name="psr4")
                        for ep in range(EPC):
                            for c in range(NC):
                                nc.tensor.matmul(
                                    psr4[:, ep, :], s16_sb[:, ep, c, es],
                                    aT_sb[:, ep, c, :],
                                    start=(ep == 0 and c == 0),
                                    stop=(ep == EPC - 1 and c == NC - 1))
                    # C' = 0.5*(t_f+1)*C + (t_i+1)*t_g   (C = 2c). The last
                    # chunk runs ep-sliced so the next step's dots/gates (which
                    # need the full u8) start ~2.5us earlier.
                    if e < EC - 1:
                        slices = [slice(None)]
                    else:
                        slices = [slice(ep * 128, (ep + 1) * 128)
                                  for ep in range(EPC)]
                    for si, cs2 in enumerate(slices):
                        av = ptmp.tile([128, R], FP16, tag="tmp", name="av")
                        avs = av[:, :R // len(slices)]
                        nc.vector.scalar_tensor_tensor(
                            avs, tg[:, 1, cs2], 1.0, cC[:, e, cs2],
                            op0=ALU.add, op1=ALU.mult)
                        bg = ptmp.tile([128, R], FP16, tag="tmp", name="bg")
                        bgs = bg[:, :R // len(slices)]
                        nc.gpsimd.scalar_tensor_tensor(
                            bgs, tg[:, 0, cs2], 1.0, tg[:, 2, cs2],
                            op0=ALU.add, op1=ALU.mult)
                        nc.vector.scalar_tensor_tensor(
                            cC[:, e, cs2], avs, 0.5, bgs, op0=ALU.mult,
                            op1=ALU.add)
                        thc = pth.tile([128, R], FP16, tag="th")
                        thcs = thc[:, :R // len(slices)]
                        nc.scalar.activation(thcs, cC[:, e, cs2], AF.Tanh,
                                             scale=0.5)
                        eng = nc.vector if len(slices) > 1 else nc.gpsimd
                        eng.scalar_tensor_tensor(A2[:, e, cs2], tg[:, 3, cs2],
                                                 1.0, thcs, op0=ALU.add,
                                                 op1=ALU.mult)
                        if not last:
                            # u(t+1) = 0.5*A2 + r(t)
                            if len(slices) > 1:
                                rsrc = psr4[:, si, :]
                            else:
                                rsrc = psr4.rearrange("p a b -> p (a b)")
                            nc.vector.scalar_tensor_tensor(
                                u8nxt[:, e, cs2], A2[:, e, cs2], 0.5, rsrc,
                                op0=ALU.mult, op1=ALU.add)
                        else:
                            nc.scalar.dma_start(out=a2o[:, e, cs2],
                                                in_=A2[:, e, cs2])
                    if t == 1:
                        nc.gpsimd.tensor_copy(r1_sb[:, e, :],
                                              psr4.rearrange("p a b -> p (a b)"))
                if dbg:
                    nc.sync.dma_start(out=dbg[f"a{t}"], in_=A2[:])
                    if not last:
                        nc.sync.dma_start(out=dbg[f"u{t}"], in_=u8nxt[:])

    nc.compile()
    return nc


def _get_nc(n_steps=KS):
    if n_steps not in _STATE:
        _STATE[n_steps] = _build(n_steps)
    return _STATE[n_steps]


def _chunk_block(w):
    """[128, KP, 2, G] fp8 -> [EC, 128, KP, 2, 512] with chunk e covering
    gate tiles (e, 8+e, 16+e, 24+e)."""
    out = np.empty((EC, 128, KP, 2, 512), dtype=w.dtype)
    for e in range(EC):
        cols = np.concatenate([np.arange(m * 128, (m + 1) * 128)
                               for m in (e, 8 + e, 16 + e, 24 + e)])
        out[e] = w[:, :, :, cols]
    return out


def _prep_in_maps(targets, support_embeddings, W_ih, W_hh, b_ih, b_hh):
    targets = np.asarray(targets, np.float32)
    S_all = np.asarray(support_embeddings, np.float32)
    W_ih = np.asarray(W_ih, np.float32)
    W_hh = np.asarray(W_hh, np.float32)
    b = np.asarray(b_ih, np.float32) + np.asarray(b_hh, np.float32)

    rs = np.full(G, 0.5, np.float32)
    rs[2 * E:3 * E] = 1.0
    Ws_hh = W_hh * rs[:, None] * LAM
    Ws_ih = W_ih * rs[:, None] * LAM

    W8h = Ws_hh.astype(F8NP)
    W8l = (Ws_hh - W8h.astype(np.float32)).astype(F8NP)
    kpair = lambda w: np.ascontiguousarray(
        w.T.reshape(KP, 2, 128, G).transpose(2, 0, 1, 3))   # [128, KP, 2, G]
    w8h_np = _chunk_block(kpair(W8h))
    w8l_np = _chunk_block(kpair(W8l))

    wih_f = np.ascontiguousarray(
        Ws_ih.reshape(GT, 128, EC, 128).transpose(0, 3, 2, 1))
    bias_t = np.ascontiguousarray((b * rs).reshape(GT, 128).T)
    bias_z = np.ascontiguousarray((b * rs * LAM).reshape(GT, 128).T)

    i8_np = np.zeros((128, 2, 128), dtype=F8NP)
    eye = np.eye(128, dtype=np.float32)
    i8_np[:, 0, :] = eye.astype(F8NP)
    i8_np[:, 1, :] = eye.astype(F8NP)
    i16_np = eye.astype(F16NP)

    in_maps = []
    for i in range(CORES):
        x = targets[EPC * i:EPC * (i + 1)].reshape(R, E)
        S = S_all[EPC * i:EPC * (i + 1)].reshape(EPC, NK, E)
        xT = np.ascontiguousarray(x.T)                      # [E, R]
        x8h = xT.astype(F8NP)
        x8l = (xT - x8h.astype(np.float32)).astype(F8NP)

        Spad = np.zeros((EPC, NKP, E), np.float32)
        Spad[:, :NK, :] = S
        stf_np = np.ascontiguousarray(Spad.transpose(0, 2, 1))  # [EPC, E, NKP]
        r0 = S.mean(axis=1)                                  # [EPC, E]
        in_maps.append({
            "wih": wih_f,
            "w8h": w8h_np,
            "w8l": w8l_np,
            "x8": np.stack([x8h, x8l], axis=0),
            "xtf": xT,
            "bias_t": bias_t,
            "bias_z": bias_z,
            "stf": stf_np,
            "s8d": stf_np.astype(F8NP),
            "s16": np.ascontiguousarray(
                Spad.reshape(EPC, NC, 128, E)).astype(F16NP),
            "r0": np.ascontiguousarray(r0.T.reshape(EC, 128, EPC),
                                       dtype=np.float32),
            "i8": i8_np,
            "i16": i16_np,
        })
    return in_maps


def _finish(a2, x_core):
    """h = 0.5*A2 + x on the host (x exact in f32)."""
    uoT = np.asarray(a2, dtype=np.float32).transpose(1, 0, 2).reshape(E, R) * 0.5
    return (uoT.T + x_core.reshape(R, E)).reshape(EPC, T, E)


def kernel(**inputs):
    nc = _get_nc()
    inputs = {k: np.asarray(v) for k, v in inputs.items()}
    in_maps = _prep_in_maps(**inputs)
    res = run_bass_kernel_spmd(nc, in_maps, core_ids=list(range(CORES)))
    targets = np.asarray(inputs["targets"], np.float32)
    out = np.empty((B, T, E), dtype=np.float32)
    for i in range(CORES):
        out[EPC * i:EPC * (i + 1)] = _finish(res.results[i]["a2o"],
                                             targets[EPC * i:EPC * (i + 1)])
    return out


if __name__ == "__main__":
    nc = _get_nc()
    print("build+compile OK; instructions:",
          sum(len(b.instructions) for f in nc.m.functions for b in f.blocks))


# revision 22
# speedup vs baseline: 1.0319x; 1.0319x over previous
"""Trainium2 Bass kernel for the 8-step attentive LSTM ("read-process"
FLayer): B=32, T=128, E=1024, N*k=320 support vectors, K_STEPS=8.

Sharding: data-parallel over B across 8 NeuronCores (4 episodes/core), LSTM
weights replicated, the sequential K loop stays local per core. No collectives.

The dominant matmuls run in fp8-e4m3 DoubleRow mode (two 128-deep contraction
chunks per instruction at 0.5 cycles/row - 4x the bf16 rate) with a precision
scheme validated against the f32 reference in numpy (rel_err 8.3e-3, budget
2e-2; the bf16 baseline measured 1.3e-3 at 582us):

  * u-form recurrence: u(t) = o*tanh(c) + r(t-1) (= h_in - x), so the
    per-step matmul is gates = W_hh.u + Z with Z = (W_ih+W_hh).x + b
    precomputed once; the x part of the moving operand is never
    re-quantized (measurably better than quantizing h = u + x).
  * weights/Z pre-scaled by LAM=16 (escapes fp8 subnormals; i,f,o rows also
    fold the sigmoid-via-tanh 0.5), descaled for free by the ACT tanh input
    scale.  sigmoid(x) = 0.5 tanh(x/2)+0.5 is fused into scalar_tensor_tensor
    ops on a doubled cell state C = 2c, so the cell costs 2 batched ACT tanh
    + 1 tanh(C/2) + 4 fused DVE/Pool ops per E-chunk, and every ACT func
    (tanh/exp/identity) lives in the single "exp_and_others" table.
  * 2-limb fp8 (value + unscaled residual; the residual lands in fp8's
    subnormal range where absolute error beats bf16): W_hh on odd steps, Z
    always (added inside the matmul accumulation by an fp8 [I,I] DoubleRow
    pair straight from the resident 2-limb Z), dots_x likewise.
  * chaos containment: step-0 gates and step-1/2 logits in exact f32 (f32r
    matmuls run at bf16 speed), r(t) through a fp16 S copy at every step
    (fp16 r-matmul = 1.0 cy/row, precision ~0.05%); from step 3 the softmax
    is saturated (top-gap ~800) and logits run 1-limb fp8.
  * the final h = 0.5*A2 + x add runs on the host (x exact there); the
    device returns A2 = (t_o+1)*tanh(c) in fp16.

Scheduling: gate PSUM is double-buffered in half-chunks (i,f | g,o) so PE
never waits on the ACT drain; per E-chunk the r-matmuls of all 4 episodes
land in one PSUM bank so u(t+1) assembly is a single fused stt; attention
r-matmuls are interleaved into the gates loop (after the chunk's gates) to
keep the in-order PE queue deadlock-free with 2 PSUM bufs.
"""

import os
import sys

for _p in ("/opt/trn_rl_repo", "/root/.axon_site/_ro/trn_rl_repo"):
    if os.path.isdir(_p) and _p not in sys.path:
        sys.path.insert(0, _p)

import numpy as np
import ml_dtypes

import concourse.bass as bass
import concourse.mybir as mybir
import concourse.tile as tile
from concourse import bacc
from concourse.bass_utils import run_bass_kernel_spmd

# Problem shape (hardcoded per contract)
B, T, E, NK, KS = 32, 128, 1024, 320, 8
CORES = 8
EPC = B // CORES          # episodes per core = 4
R = EPC * T               # rows per core = 512
G = 4 * E                 # gate rows = 4096
EC = E // 128             # E chunks = 8
GT = G // 128             # gate tiles = 32
KP = EC // 2              # contraction pairs = 4
NKP = 384                 # padded support count (3 chunks of 128)
NC = NKP // 128           # nk chunks = 3
LAM = 16.0                # weight prescale (fp8 subnormal escape)
WLO_STEPS = (1, 3, 5, 7)  # steps that add the W_hh low limb

F32 = mybir.dt.float32
F32R = mybir.dt.float32r
BF16 = mybir.dt.bfloat16
FP16 = mybir.dt.float16
FP8 = mybir.dt.float8e4
AF = mybir.ActivationFunctionType
ALU = mybir.AluOpType
AX = mybir.AxisListType
DR = mybir.MatmulPerfMode.DoubleRow
F8NP = ml_dtypes.float8_e4m3
F16NP = np.float16

_STATE = {}


def _build(n_steps=KS):
    nc = bacc.Bacc("TRN2", target_bir_lowering=False, debug=False,
                   enable_asserts=True)

    # ---- DRAM parameters (per-core shards; host preps layouts/dtypes) ----
    wih = nc.dram_tensor("wih", [GT, 128, EC, 128], F32R, kind="ExternalInput").ap()
    w8h = nc.dram_tensor("w8h", [EC, 128, KP, 2, 512], FP8, kind="ExternalInput").ap()
    w8l = nc.dram_tensor("w8l", [EC, 128, KP, 2, 512], FP8, kind="ExternalInput").ap()
    x8 = nc.dram_tensor("x8", [2, E, R], FP8, kind="ExternalInput").ap()
    xtf = nc.dram_tensor("xtf", [E, R], F32R, kind="ExternalInput").ap()
    bt_d = nc.dram_tensor("bias_t", [128, GT], F32, kind="ExternalInput").ap()
    bz_d = nc.dram_tensor("bias_z", [128, GT], F32, kind="ExternalInput").ap()
    stf = nc.dram_tensor("stf", [EPC, E, NKP], F32R, kind="ExternalInput").ap()
    s8d = nc.dram_tensor("s8d", [EPC, E, NKP], FP8, kind="ExternalInput").ap()
    s16 = nc.dram_tensor("s16", [EPC, NC, 128, E], FP16, kind="ExternalInput").ap()
    r0d = nc.dram_tensor("r0", [EC, 128, EPC], F32, kind="ExternalInput").ap()
    i8d = nc.dram_tensor("i8", [128, 2, 128], FP8, kind="ExternalInput").ap()
    i16d = nc.dram_tensor("i16", [128, 128], FP16, kind="ExternalInput").ap()
    a2o = nc.dram_tensor("a2o", [128, EC, R], FP16, kind="ExternalOutput").ap()
    dbg = {}
    if os.environ.get("K_DEBUG"):
        for t in range(KS):
            dbg[f"u{t}"] = nc.dram_tensor(f"dbg_u{t}", [128, EC, R], FP8,
                                          kind="ExternalOutput").ap()
            dbg[f"a{t}"] = nc.dram_tensor(f"dbg_a{t}", [128, EC, R], FP16,
                                          kind="ExternalOutput").ap()

    with tile.TileContext(nc) as tc:
        with (
            tc.tile_pool(name="res", bufs=1) as res,
            tc.tile_pool(name="pxw", bufs=4) as pxw,
            tc.tile_pool(name="pw8l", bufs=2) as pw8l,
            tc.tile_pool(name="pgw", bufs=2) as pgw,
            tc.tile_pool(name="ptmp", bufs=3) as ptmp,
            tc.tile_pool(name="pth", bufs=2) as pth,
            tc.tile_pool(name="pbig", bufs=2) as pbig,
            tc.tile_pool(name="pat", bufs=2) as pat,
            tc.tile_pool(name="pstat", bufs=8) as pstat,
            tc.tile_pool(name="ppg", bufs=2, space="PSUM") as ppg_pool,
            tc.tile_pool(name="pp1", bufs=2, space="PSUM") as pp1,
            tc.tile_pool(name="pp2", bufs=2, space="PSUM") as pp2,
        ):
            # ---- resident tiles ----
            w8h_sb = res.tile([128, EC, KP, 2, 512], FP8, tag="w8h")
            z8_sb = res.tile([128, EC, 2, 4, R], FP8, tag="z8")
            x8h_sb = res.tile([128, EC, R], FP8, tag="x8h")
            x8l_sb = res.tile([128, EC, R], FP8, tag="x8l")
            xtf_sb = res.tile([128, EC, R], F32R, tag="xtf")
            s8d_sb = res.tile([128, EPC, EC, NKP], FP8, tag="s8d")
            s16_sb = res.tile([128, EPC, NC, E], FP16, tag="s16")
            dx8_sb = res.tile([128, EPC, 2, NKP], FP8, tag="dx8")
            aT_sb = res.tile([128, EPC, NC, 128], FP16, tag="aT")
            r1_sb = res.tile([128, EC, R], FP16, tag="r1")
            u8a = res.tile([128, EC, R], FP8, tag="u8a")
            u8b = res.tile([128, EC, R], FP8, tag="u8b")
            cC = res.tile([128, EC, R], FP16, tag="cC")     # doubled cell 2c
            A2 = res.tile([128, EC, R], FP16, tag="A2")     # (t_o+1)*tanh(c)
            bt_sb = res.tile([128, GT], F32, tag="bt")
            bz_sb = res.tile([128, GT], F32, tag="bz")
            r0_sb = res.tile([128, EC, EPC], F32, tag="r0")
            i8_sb = res.tile([128, 2, 128], FP8, tag="i8")
            i16_sb = res.tile([128, 128], FP16, tag="i16")

            # resident loads: what the first zbuild/Xp tiles need comes first
            # (x8 limbs + w8h block 0 on ACT queue, xtf on SP); the rest
            # trickles in behind.
            nc.scalar.dma_start(out=x8h_sb[:],
                                in_=x8[0].rearrange("(k p) c -> p k c", p=128))
            nc.scalar.dma_start(out=x8l_sb[:],
                                in_=x8[1].rearrange("(k p) c -> p k c", p=128))
            nc.sync.dma_start(out=xtf_sb[:],
                              in_=xtf.rearrange("(k p) c -> p k c", p=128))
            for e in range(EC):
                nc.scalar.dma_start(out=w8h_sb[:, e, :, :, :], in_=w8h[e])
            nc.scalar.dma_start(out=bt_sb[:], in_=bt_d[:])
            nc.scalar.dma_start(out=bz_sb[:], in_=bz_d[:])
            nc.scalar.dma_start(out=r0_sb[:], in_=r0d.rearrange("k p e -> p k e"))

            def u8pair(u8t, kp, cols=slice(None)):
                return u8t[:, 2 * kp:2 * kp + 2, cols]

            # ================= phase B =================
            # Z = (W_ih + W_hh).x + b, prescaled by LAM*rs.  W_ih part exact
            # f32r (doubles as the chaos-sensitive step-0 gates); W_hh part
            # rebuilt from the resident fp8 limbs (err ~0.2%, folded into the
            # 2-limb Z storage).
            for e in range(EC):
                ms = (e, 8 + e, 16 + e, 24 + e)     # i, f, g, o gate tiles
                w8lt = pw8l.tile([128, KP, 2, 512], FP8, tag="w8l")
                (nc.sync if e % 2 == 0 else nc.scalar).dma_start(
                    out=w8lt[:], in_=w8l[e])
                tg0 = pgw.tile([128, 4, R], FP16, tag="gw")
                for half in range(2):
                    ppg = ppg_pool.tile([128, 2, R], F32, tag="pg")
                    for hi in range(2):
                        gi = 2 * half + hi
                        m = ms[gi]
                        # fp8 W_hh.x limb products
                        zps = pp1.tile([128, R], F32, tag="p1")
                        first = True
                        for pi, (hi_limb, xt) in enumerate(
                                ((True, x8h_sb), (False, x8h_sb),
                                 (True, x8l_sb))):
                            wt = w8h_sb[:, e] if hi_limb else w8lt
                            for kp in range(KP):
                                nc.tensor.matmul(
                                    zps[:],
                                    wt[:, kp, :, gi * 128:(gi + 1) * 128],
                                    u8pair(xt, kp), start=first,
                                    stop=(pi == 2 and kp == KP - 1),
                                    perf_mode=DR)
                                first = False
                        zbt = ptmp.tile([128, R], FP16, tag="tmp", name="zb")
                        nc.scalar.activation(zbt[:], zps[:], AF.Identity)
                        # exact f32r W_ih.x into the gate slot; the weight
                        # stream rides both HWDGE queues in half tiles
                        hf = EC // 2
                        xwa = pxw.tile([128, hf, 128], F32R, tag="xw",
                                       name="xwa")
                        xwb = pxw.tile([128, hf, 128], F32R, tag="xw",
                                       name="xwb")
                        nc.sync.dma_start(out=xwa[:], in_=wih[m][:, :hf, :])
                        nc.scalar.dma_start(out=xwb[:], in_=wih[m][:, hf:, :])
                        for k in range(EC):
                            lhs = xwa[:, k, :] if k < hf else xwb[:, k - hf, :]
                            nc.tensor.matmul(ppg[:, hi, :], lhs,
                                             xtf_sb[:, k, :], start=(k == 0),
                                             stop=(k == EC - 1))
                        # step-0 gate tanh + Z limb assembly
                        nc.scalar.activation(tg0[:, gi, :], ppg[:, hi, :],
                                             AF.Tanh, bias=bt_sb[:, m:m + 1],
                                             scale=1.0 / LAM)
                        zf = ptmp.tile([128, R], FP16, tag="tmp", name="zf")
                        nc.vector.scalar_tensor_tensor(
                            zf[:], ppg[:, hi, :], bz_sb[:, m:m + 1], zbt[:],
                            op0=ALU.add, op1=ALU.add)
                        nc.gpsimd.tensor_copy(z8_sb[:, e, 0, gi, :], zf[:])
                        nc.vector.tensor_sub(z8_sb[:, e, 1, gi, :], zf[:],
                                             z8_sb[:, e, 0, gi, :])
                # step-0 cell (h=c=r=0): C0 = (t_i+1)*t_g; A2 = (t_o+1)tanh(C0/2)
                nc.gpsimd.scalar_tensor_tensor(cC[:, e, :], tg0[:, 0, :], 1.0,
                                               tg0[:, 2, :], op0=ALU.add,
                                               op1=ALU.mult)
                thc = pth.tile([128, R], FP16, tag="th")
                nc.scalar.activation(thc[:], cC[:, e, :], AF.Tanh, scale=0.5)
                nc.gpsimd.scalar_tensor_tensor(A2[:, e, :], tg0[:, 3, :], 1.0,
                                               thc[:], op0=ALU.add, op1=ALU.mult)
                # u(1) = 0.5*A2 + r0 (r0 = per-episode support mean, exact)
                for ep in range(EPC):
                    cs = slice(ep * 128, (ep + 1) * 128)
                    nc.gpsimd.tensor_scalar(u8a[:, e, cs], A2[:, e, cs], 0.5,
                                            r0_sb[:, e, ep:ep + 1],
                                            op0=ALU.mult, op1=ALU.add)
                    # r1 staging doubles as the q-assembly source: x + r0 for
                    # step-1 logits (overwritten with r(1)+x during step 1)
                    nc.gpsimd.tensor_scalar_add(
                        r1_sb[:, e, cs], xtf_sb[:, e, cs].bitcast(F32),
                        r0_sb[:, e, ep:ep + 1])
            # loads first needed from step 1 on, deferred so the phase-B
            # weight stream owns the queues
            nc.scalar.dma_start(out=i8_sb[:], in_=i8d[:])
            nc.scalar.dma_start(out=i16_sb[:], in_=i16d[:])
            nc.scalar.dma_start(out=s16_sb[:],
                                in_=s16.rearrange("ep c p e -> p ep c e"))
            nc.scalar.dma_start(out=s8d_sb[:],
                                in_=s8d.rearrange("ep (k p) c -> p ep k c", p=128))
            if dbg:
                nc.sync.dma_start(out=dbg["u0"], in_=u8a[:])
                nc.sync.dma_start(out=dbg["a0"], in_=A2[:])

            # ================= steps 1..7 =================
            for t in range(1, n_steps):
                last = (t == n_steps - 1)
                wlo = t in WLO_STEPS
                u8cur, u8nxt = (u8a, u8b) if t % 2 == 1 else (u8b, u8a)

                # ---- attention front: logits/softmax/transposes per ep ----
                if not last:
                    for ep in range(EPC):
                        cs = slice(ep * 128, (ep + 1) * 128)
                        psd = pp1.tile([128, NKP], F32, tag="p1", name="psd")
                        if t <= 2:
                            # exact f32r logits (steps 1-2 are chaos-sensitive)
                            half = EC // 2
                            sview = stf[ep].rearrange("(k p) c -> p k c", p=128)
                            spa = pbig.tile([128, half, NKP], F32R, tag="big",
                                            name="spa")
                            spb = pbig.tile([128, half, NKP], F32R, tag="big",
                                            name="spb")
                            nc.sync.dma_start(out=spa[:], in_=sview[:, :half, :])
                            nc.scalar.dma_start(out=spb[:], in_=sview[:, half:, :])
                            # q tiles overlay the x8 limb tiles (dead after
                            # phase B): fp8 [128,EC,512] viewed as f32
                            # [128,EC,128], ping-ponged across episodes.
                            qt = (x8h_sb if ep % 2 == 0 else
                                  x8l_sb)[:].bitcast(F32)
                            # q(t) = 0.5*A2(t-1) + (x + r(t-1)) with the x+r
                            # part staged fp16 in r1_sb (x+r0 from phase B at
                            # t=1, r(1)+x written during step 1 for t=2)
                            nc.vector.scalar_tensor_tensor(
                                qt[:], A2[:, :, cs], 0.5, r1_sb[:, :, cs],
                                op0=ALU.mult, op1=ALU.add)
                            for k in range(EC):
                                rhs = (spa[:, k, :] if k < half
                                       else spb[:, k - half, :])
                                nc.tensor.matmul(psd[:],
                                                 qt[:, k, :].bitcast(F32R), rhs,
                                                 start=(k == 0),
                                                 stop=(k == EC - 1))
                            if t == 2:
                                # dots_x = x.S exact -> fp8 limbs for steps 3+
                                dxp = pp1.tile([128, NKP], F32, tag="p1",
                                               name="dxp")
                                for k in range(EC):
                                    rhs = (spa[:, k, :] if k < half
                                           else spb[:, k - half, :])
                                    nc.tensor.matmul(dxp[:], xtf_sb[:, k, cs],
                                                     rhs, start=(k == 0),
                                                     stop=(k == EC - 1))
                                nc.scalar.activation(dx8_sb[:, ep, 0, :],
                                                     dxp[:], AF.Identity)
                                nc.vector.tensor_sub(dx8_sb[:, ep, 1, :],
                                                     dxp[:],
                                                     dx8_sb[:, ep, 0, :])
                        else:
                            # fp8 logits: dots_x pair + u8.S8h
                            nc.tensor.matmul(psd[:], i8_sb[:],
                                             dx8_sb[:, ep, :, :], start=True,
                                             stop=False, perf_mode=DR)
                            for kp in range(KP):
                                nc.tensor.matmul(
                                    psd[:], u8pair(u8cur, kp, cs),
                                    s8d_sb[:, ep, 2 * kp:2 * kp + 2, :],
                                    start=False, stop=(kp == KP - 1),
                                    perf_mode=DR)
                        # softmax over NKP (padding exps to ~0 automatically)
                        nmax = pstat.tile([128, 1], F32, tag="st")
                        nc.vector.tensor_reduce(nmax[:], psd[:], axis=AX.X,
                                                op=ALU.max, negate=True)
                        ate = pat.tile([128, NKP], FP16, tag="ate")
                        sm = pstat.tile([128, 1], F32, tag="st")
                        nc.scalar.activation(ate[:], psd[:], AF.Exp,
                                             bias=nmax[:, :], accum_out=sm[:, :])
                        rec = pstat.tile([128, 1], F32, tag="st")
                        nc.vector.reciprocal(rec[:], sm[:])
                        at16 = pat.tile([128, NKP], FP16, tag="at16")
                        nc.gpsimd.tensor_scalar_mul(at16[:], ate[:], rec[:, :])
                        for c in range(NC):
                            pt = pp2.tile([128, 128], FP16, tag="p2", name="pt")
                            nc.tensor.transpose(
                                pt[:], at16[:, c * 128:(c + 1) * 128],
                                i16_sb[:, :])
                            nc.vector.tensor_copy(aT_sb[:, ep, c, :], pt[:])

                # ---- gates + cell update + interleaved r-matmuls ----
                for e in range(EC):
                    zt = z8_sb[:, e]
                    if wlo:
                        w8lt = pw8l.tile([128, KP, 2, 512], FP8, tag="w8l")
                        (nc.scalar if (last or e % 2) else nc.sync).dma_start(
                            out=w8lt[:], in_=w8l[e])
                    tg = pgw.tile([128, 4, R], FP16, tag="gw")
                    for half in range(2):
                        ppg = ppg_pool.tile([128, 2, R], F32, tag="pg")
                        for hi in range(2):
                            gi = 2 * half + hi
                            slot = ppg[:, hi, :]
                            first = True
                            if wlo:
                                for kp in range(KP):
                                    nc.tensor.matmul(
                                        slot,
                                        w8lt[:, kp, :, gi * 128:(gi + 1) * 128],
                                        u8pair(u8cur, kp), start=first,
                                        stop=False, perf_mode=DR)
                                    first = False
                            for kp in range(KP):
                                nc.tensor.matmul(
                                    slot,
                                    w8h_sb[:, e, kp, :, gi * 128:(gi + 1) * 128],
                                    u8pair(u8cur, kp), start=first, stop=False,
                                    perf_mode=DR)
                                first = False
                            nc.tensor.matmul(slot, i8_sb[:], zt[:, :, gi, :],
                                             start=False, stop=True,
                                             perf_mode=DR)
                        nc.scalar.activation(tg[:, 2 * half:2 * half + 2, :],
                                             ppg[:], AF.Tanh, scale=1.0 / LAM)
                    # r-matmuls for this E-chunk: all 4 episodes into one
                    # PSUM bank (fp16 S, one accumulation group)
                    psr4 = None
                    if not last:
                        es = slice(e * 128, (e + 1) * 128)
                        psr4 = pp2.tile([128, EPC, 128], F32, tag="p2",
                                        name="psr4")
                        for ep in range(EPC):
                            for c in range(NC):
                                nc.tensor.matmul(
                                    psr4[:, ep, :], s16_sb[:, ep, c, es],
                                    aT_sb[:, ep, c, :],
                                    start=(ep == 0 and c == 0),
                                    stop=(ep == EPC - 1 and c == NC - 1))
                    # C' = 0.5*(t_f+1)*C + (t_i+1)*t_g   (C = 2c). The last
                    # chunk runs ep-sliced so the next step's dots/gates (which
                    # need the full u8) start ~2.5us earlier.
                    if e < EC - 1:
                        slices = [slice(None)]
                    else:
                        slices = [slice(ep * 128, (ep + 1) * 128)
                                  for ep in range(EPC)]
                    for si, cs2 in enumerate(slices):
                        av = ptmp.tile([128, R], FP16, tag="tmp", name="av")
                        avs = av[:, :R // len(slices)]
                        nc.vector.scalar_tensor_tensor(
                            avs, tg[:, 1, cs2], 1.0, cC[:, e, cs2],
                            op0=ALU.add, op1=ALU.mult)
                        bg = ptmp.tile([128, R], FP16, tag="tmp", name="bg")
                        bgs = bg[:, :R // len(slices)]
                        nc.gpsimd.scalar_tensor_tensor(
                            bgs, tg[:, 0, cs2], 1.0, tg[:, 2, cs2],
                            op0=ALU.add, op1=ALU.mult)
                        nc.vector.scalar_tensor_tensor(
                            cC[:, e, cs2], avs, 0.5, bgs, op0=ALU.mult,
                            op1=ALU.add)
                        thc = pth.tile([128, R], FP16, tag="th")
                        thcs = thc[:, :R // len(slices)]
                        nc.scalar.activation(thcs, cC[:, e, cs2], AF.Tanh,
                                             scale=0.5)
                        eng = nc.vector if len(slices) > 1 else nc.gpsimd
                        eng.scalar_tensor_tensor(A2[:, e, cs2], tg[:, 3, cs2],
                                                 1.0, thcs, op0=ALU.add,
                                                 op1=ALU.mult)
                        if not last:
                            # u(t+1) = 0.5*A2 + r(t)
                            if len(slices) > 1:
                                rsrc = psr4[:, si, :]
                            else:
                                rsrc = psr4.rearrange("p a b -> p (a b)")
                            nc.vector.scalar_tensor_tensor(
                                u8nxt[:, e, cs2], A2[:, e, cs2], 0.5, rsrc,
                                op0=ALU.mult, op1=ALU.add)
                        else:
                            nc.sync.dma_start(out=a2o[:, e, cs2],
                                              in_=A2[:, e, cs2])
                    if t == 1:
                        nc.gpsimd.scalar_tensor_tensor(
                            r1_sb[:, e, :],
                            psr4.rearrange("p a b -> p (a b)"), 1.0,
                            xtf_sb[:, e, :].bitcast(F32),
                            op0=ALU.mult, op1=ALU.add)
                if dbg:
                    nc.sync.dma_start(out=dbg[f"a{t}"], in_=A2[:])
                    if not last:
                        nc.sync.dma_start(out=dbg[f"u{t}"], in_=u8nxt[:])

    nc.compile()
    return nc


def _get_nc(n_steps=KS):
    if n_steps not in _STATE:
        _STATE[n_steps] = _build(n_steps)
    return _STATE[n_steps]


def _chunk_block(w):
    """[128, KP, 2, G] fp8 -> [EC, 128, KP, 2, 512] with chunk e covering
    gate tiles (e, 8+e, 16+e, 24+e)."""
    out = np.empty((EC, 128, KP, 2, 512), dtype=w.dtype)
    for e in range(EC):
        cols = np.concatenate([np.arange(m * 128, (m + 1) * 128)
                               for m in (e, 8 + e, 16 + e, 24 + e)])
        out[e] = w[:, :, :, cols]
    return out


def _prep_in_maps(targets, support_embeddings, W_ih, W_hh, b_ih, b_hh):
    targets = np.asarray(targets, np.float32)
    S_all = np.asarray(support_embeddings, np.float32)
    W_ih = np.asarray(W_ih, np.float32)
    W_hh = np.asarray(W_hh, np.float32)
    b = np.asarray(b_ih, np.float32) + np.asarray(b_hh, np.float32)

    rs = np.full(G, 0.5, np.float32)
    rs[2 * E:3 * E] = 1.0
    Ws_hh = W_hh * rs[:, None] * LAM
    Ws_ih = W_ih * rs[:, None] * LAM

    W8h = Ws_hh.astype(F8NP)
    W8l = (Ws_hh - W8h.astype(np.float32)).astype(F8NP)
    kpair = lambda w: np.ascontiguousarray(
        w.T.reshape(KP, 2, 128, G).transpose(2, 0, 1, 3))   # [128, KP, 2, G]
    w8h_np = _chunk_block(kpair(W8h))
    w8l_np = _chunk_block(kpair(W8l))

    wih_f = np.ascontiguousarray(
        Ws_ih.reshape(GT, 128, EC, 128).transpose(0, 3, 2, 1))
    bias_t = np.ascontiguousarray((b * rs).reshape(GT, 128).T)
    bias_z = np.ascontiguousarray((b * rs * LAM).reshape(GT, 128).T)

    i8_np = np.zeros((128, 2, 128), dtype=F8NP)
    eye = np.eye(128, dtype=np.float32)
    i8_np[:, 0, :] = eye.astype(F8NP)
    i8_np[:, 1, :] = eye.astype(F8NP)
    i16_np = eye.astype(F16NP)

    in_maps = []
    for i in range(CORES):
        x = targets[EPC * i:EPC * (i + 1)].reshape(R, E)
        S = S_all[EPC * i:EPC * (i + 1)].reshape(EPC, NK, E)
        xT = np.ascontiguousarray(x.T)                      # [E, R]
        x8h = xT.astype(F8NP)
        x8l = (xT - x8h.astype(np.float32)).astype(F8NP)

        Spad = np.zeros((EPC, NKP, E), np.float32)
        Spad[:, :NK, :] = S
        stf_np = np.ascontiguousarray(Spad.transpose(0, 2, 1))  # [EPC, E, NKP]
        r0 = S.mean(axis=1)                                  # [EPC, E]
        in_maps.append({
            "wih": wih_f,
            "w8h": w8h_np,
            "w8l": w8l_np,
            "x8": np.stack([x8h, x8l], axis=0),
            "xtf": xT,
            "bias_t": bias_t,
            "bias_z": bias_z,
            "stf": stf_np,
            "s8d": stf_np.astype(F8NP),
            "s16": np.ascontiguousarray(
                Spad.reshape(EPC, NC, 128, E)).astype(F16NP),
            "r0": np.ascontiguousarray(r0.T.reshape(EC, 128, EPC),
                                       dtype=np.float32),
            "i8": i8_np,
            "i16": i16_np,
        })
    return in_maps


def _finish(a2, x_core):
    """h = 0.5*A2 + x on the host (x exact in f32)."""
    uoT = np.asarray(a2, dtype=np.float32).transpose(1, 0, 2).reshape(E, R) * 0.5
    return (uoT.T + x_core.reshape(R, E)).reshape(EPC, T, E)


def kernel(**inputs):
    nc = _get_nc()
    inputs = {k: np.asarray(v) for k, v in inputs.items()}
    in_maps = _prep_in_maps(**inputs)
    res = run_bass_kernel_spmd(nc, in_maps, core_ids=list(range(CORES)))
    targets = np.asarray(inputs["targets"], np.float32)
    out = np.empty((B, T, E), dtype=np.float32)
    for i in range(CORES):
        out[EPC * i:EPC * (i + 1)] = _finish(res.results[i]["a2o"],
                                             targets[EPC * i:EPC * (i + 1)])
    return out


if __name__ == "__main__":
    nc = _get_nc()
    print("build+compile OK; instructions:",
          sum(len(b.instructions) for f in nc.m.functions for b in f.blocks))


# revision 27
# speedup vs baseline: 1.0340x; 1.0021x over previous
"""Trainium2 Bass kernel for the 8-step attentive LSTM ("read-process"
FLayer): B=32, T=128, E=1024, N*k=320 support vectors, K_STEPS=8.

Sharding: data-parallel over B across 8 NeuronCores (4 episodes/core), LSTM
weights replicated, the sequential K loop stays local per core. No collectives.

The dominant matmuls run in fp8-e4m3 DoubleRow mode (two 128-deep contraction
chunks per instruction at 0.5 cycles/row - 4x the bf16 rate) with a precision
scheme validated against the f32 reference in numpy (rel_err 8.3e-3, budget
2e-2; the bf16 baseline measured 1.3e-3 at 582us):

  * u-form recurrence: u(t) = o*tanh(c) + r(t-1) (= h_in - x), so the
    per-step matmul is gates = W_hh.u + Z with Z = (W_ih+W_hh).x + b
    precomputed once; the x part of the moving operand is never
    re-quantized (measurably better than quantizing h = u + x).
  * weights/Z pre-scaled by LAM=16 (escapes fp8 subnormals; i,f,o rows also
    fold the sigmoid-via-tanh 0.5), descaled for free by the ACT tanh input
    scale.  sigmoid(x) = 0.5 tanh(x/2)+0.5 is fused into scalar_tensor_tensor
    ops on a doubled cell state C = 2c, so the cell costs 2 batched ACT tanh
    + 1 tanh(C/2) + 4 fused DVE/Pool ops per E-chunk, and every ACT func
    (tanh/exp/identity) lives in the single "exp_and_others" table.
  * 2-limb fp8 (value + unscaled residual; the residual lands in fp8's
    subnormal range where absolute error beats bf16): W_hh on odd steps, Z
    always (added inside the matmul accumulation by an fp8 [I,I] DoubleRow
    pair straight from the resident 2-limb Z), dots_x likewise.
  * chaos containment: step-0 gates and step-1/2 logits in exact f32 (f32r
    matmuls run at bf16 speed), r(t) through a fp16 S copy at every step
    (fp16 r-matmul = 1.0 cy/row, precision ~0.05%); from step 3 the softmax
    is saturated (top-gap ~800) and logits run 1-limb fp8.
  * the final h = 0.5*A2 + x add runs on the host (x exact there); the
    device returns A2 = (t_o+1)*tanh(c) in fp16.

Scheduling: gate PSUM is double-buffered in half-chunks (i,f | g,o) so PE
never waits on the ACT drain; per E-chunk the r-matmuls of all 4 episodes
land in one PSUM bank so u(t+1) assembly is a single fused stt; attention
r-matmuls are interleaved into the gates loop (after the chunk's gates) to
keep the in-order PE queue deadlock-free with 2 PSUM bufs.
"""

import os
import sys

for _p in ("/opt/trn_rl_repo", "/root/.axon_site/_ro/trn_rl_repo"):
    if os.path.isdir(_p) and _p not in sys.path:
        sys.path.insert(0, _p)

import numpy as np
import ml_dtypes

import concourse.bass as bass
import concourse.mybir as mybir
import concourse.tile as tile
from concourse import bacc
from concourse.bass_utils import run_bass_kernel_spmd

# Problem shape (hardcoded per contract)
B, T, E, NK, KS = 32, 128, 1024, 320, 8
CORES = 8
EPC = B // CORES          # episodes per core = 4
R = EPC * T               # rows per core = 512
G = 4 * E                 # gate rows = 4096
EC = E // 128             # E chunks = 8
GT = G // 128             # gate tiles = 32
KP = EC // 2              # contraction pairs = 4
NKP = 384                 # padded support count (3 chunks of 128)
NC = NKP // 128           # nk chunks = 3
LAM = 16.0                # weight prescale (fp8 subnormal escape)
WLO_STEPS = (1, 3, 5, 7)  # steps that add the W_hh low limb

F32 = mybir.dt.float32
F32R = mybir.dt.float32r
BF16 = mybir.dt.bfloat16
FP16 = mybir.dt.float16
FP8 = mybir.dt.float8e4
AF = mybir.ActivationFunctionType
ALU = mybir.AluOpType
AX = mybir.AxisListType
DR = mybir.MatmulPerfMode.DoubleRow
F8NP = ml_dtypes.float8_e4m3
F16NP = np.float16

_STATE = {}


def _build(n_steps=KS):
    nc = bacc.Bacc("TRN2", target_bir_lowering=False, debug=False,
                   enable_asserts=True)

    # ---- DRAM parameters (per-core shards; host preps layouts/dtypes) ----
    wih = nc.dram_tensor("wih", [GT, 128, EC, 128], FP16, kind="ExternalInput").ap()
    x16d = nc.dram_tensor("x16", [E, R], FP16, kind="ExternalInput").ap()
    w8h = nc.dram_tensor("w8h", [EC, 128, KP, 2, 512], FP8, kind="ExternalInput").ap()
    w8l = nc.dram_tensor("w8l", [EC, 128, KP, 2, 512], FP8, kind="ExternalInput").ap()
    x8 = nc.dram_tensor("x8", [2, E, R], FP8, kind="ExternalInput").ap()
    xtf = nc.dram_tensor("xtf", [E, R], F32R, kind="ExternalInput").ap()
    bt_d = nc.dram_tensor("bias_t", [128, GT], F32, kind="ExternalInput").ap()
    bz_d = nc.dram_tensor("bias_z", [128, GT], F32, kind="ExternalInput").ap()
    stf = nc.dram_tensor("stf", [EPC, E, NK], F32R, kind="ExternalInput").ap()
    s8d = nc.dram_tensor("s8d", [EPC, E, NKP], FP8, kind="ExternalInput").ap()
    s16 = nc.dram_tensor("s16", [EPC, NC, 128, E], FP16, kind="ExternalInput").ap()
    r0d = nc.dram_tensor("r0", [EC, 128, EPC], F32, kind="ExternalInput").ap()
    i8d = nc.dram_tensor("i8", [128, 2, 128], FP8, kind="ExternalInput").ap()
    i16d = nc.dram_tensor("i16", [128, 128], FP16, kind="ExternalInput").ap()
    a2o = nc.dram_tensor("a2o", [128, EC, R], FP16, kind="ExternalOutput").ap()
    dbg = {}
    if os.environ.get("K_DEBUG"):
        for t in range(KS):
            dbg[f"u{t}"] = nc.dram_tensor(f"dbg_u{t}", [128, EC, R], FP8,
                                          kind="ExternalOutput").ap()
            dbg[f"a{t}"] = nc.dram_tensor(f"dbg_a{t}", [128, EC, R], FP16,
                                          kind="ExternalOutput").ap()

    with tile.TileContext(nc) as tc:
        with (
            tc.tile_pool(name="res", bufs=1) as res,
            tc.tile_pool(name="pxw", bufs=3) as pxw,
            tc.tile_pool(name="pw8l", bufs=2) as pw8l,
            tc.tile_pool(name="pgw", bufs=2) as pgw,
            tc.tile_pool(name="ptmp", bufs=2) as ptmp,
            tc.tile_pool(name="pth", bufs=1) as pth,
            tc.tile_pool(name="pbig", bufs=2) as pbig,
            tc.tile_pool(name="pat", bufs=2) as pat,
            tc.tile_pool(name="pstat", bufs=4) as pstat,
            tc.tile_pool(name="ppg", bufs=2, space="PSUM") as ppg_pool,
            tc.tile_pool(name="pp1", bufs=2, space="PSUM") as pp1,
            tc.tile_pool(name="pp2", bufs=2, space="PSUM") as pp2,
        ):
            # ---- resident tiles ----
            w8h_sb = res.tile([128, EC, KP, 2, 512], FP8, tag="w8h")
            z8_sb = res.tile([128, EC, 2, 4, R], FP8, tag="z8")
            x8_sb = res.tile([128, 2, EC, R], FP8, tag="x8")
            x8h_sb = x8_sb[:, 0]
            x8l_sb = x8_sb[:, 1]
            x16_sb = res.tile([128, EC, R], FP16, tag="x16")
            xtf_sb = res.tile([128, EC, R], F32R, tag="xtf")
            s8d_sb = res.tile([128, EPC, EC, NKP], FP8, tag="s8d")
            s16_sb = res.tile([128, EPC, NC, E], FP16, tag="s16")
            dx8_sb = res.tile([128, EPC, 2, NKP], FP8, tag="dx8")
            aT_sb = res.tile([128, EPC, NC, 128], FP16, tag="aT")
            r1_sb = res.tile([128, EC, R], FP16, tag="r1")
            u8a = res.tile([128, EC, R], FP8, tag="u8a")
            u8b = res.tile([128, EC, R], FP8, tag="u8b")
            cC = res.tile([128, EC, R], FP16, tag="cC")     # doubled cell 2c
            A2 = res.tile([128, EC, R], FP16, tag="A2")     # (t_o+1)*tanh(c)
            bt_sb = res.tile([128, GT], F32, tag="bt")
            bz_sb = res.tile([128, GT], F32, tag="bz")
            r0_sb = res.tile([128, EC, EPC], F32, tag="r0")
            i8_sb = res.tile([128, 2, 128], FP8, tag="i8")
            i16_sb = res.tile([128, 128], FP16, tag="i16")

            # resident loads: what the first zbuild/Xp tiles need comes first
            # (x8 limbs + w8h block 0 on ACT queue, xtf on SP); the rest
            # trickles in behind.
            nc.sync.dma_start(out=x16_sb[:],
                              in_=x16d.rearrange("(k p) c -> p k c", p=128))
            nc.scalar.dma_start(out=x8_sb[:],
                                in_=x8.rearrange("j (k p) c -> p j k c", p=128))
            nc.sync.dma_start(out=xtf_sb[:],
                              in_=xtf.rearrange("(k p) c -> p k c", p=128))
            for e in range(EC):
                nc.scalar.dma_start(out=w8h_sb[:, e, :, :, :], in_=w8h[e])
            nc.scalar.dma_start(out=bt_sb[:], in_=bt_d[:])
            nc.scalar.dma_start(out=bz_sb[:], in_=bz_d[:])
            nc.scalar.dma_start(out=r0_sb[:], in_=r0d.rearrange("k p e -> p k e"))

            def u8pair(u8t, kp, cols=slice(None)):
                return u8t[:, 2 * kp:2 * kp + 2, cols]

            # ================= phase B =================
            # Z = (W_ih + W_hh).x + b, prescaled by LAM*rs.  W_ih part exact
            # f32r (doubles as the chaos-sensitive step-0 gates); W_hh part
            # rebuilt from the resident fp8 limbs (err ~0.2%, folded into the
            # 2-limb Z storage).
            for e in range(EC):
                ms = (e, 8 + e, 16 + e, 24 + e)     # i, f, g, o gate tiles
                w8lt = pw8l.tile([128, KP, 2, 512], FP8, tag="w8l")
                (nc.sync if e % 2 == 0 else nc.scalar).dma_start(
                    out=w8lt[:], in_=w8l[e])
                tg0 = pgw.tile([128, 4, R], FP16, tag="gw")
                for half in range(2):
                    ppg = ppg_pool.tile([128, 2, R], F32, tag="pg")
                    for hi in range(2):
                        gi = 2 * half + hi
                        m = ms[gi]
                        # fp16 W_ih.x into the gate slot (fp16 keeps the
                        # step-0/1 chaos path accurate enough, halves the
                        # stream, and full-tile issues amortize DMA overhead)
                        xw = pxw.tile([128, EC, 128], FP16, tag="xw")
                        (nc.sync if gi % 2 == 0 else nc.scalar).dma_start(
                            out=xw[:], in_=wih[m])
                        for k in range(EC):
                            nc.tensor.matmul(ppg[:, hi, :], xw[:, k, :],
                                             x16_sb[:, k, :], start=(k == 0),
                                             stop=(k == EC - 1))
                        # fp8 W_hh.x limb products
                        zps = pp1.tile([128, R], F32, tag="p1")
                        first = True
                        for pi, (hi_limb, xt) in enumerate(
                                ((True, x8h_sb), (False, x8h_sb),
                                 (True, x8l_sb))):
                            wt = w8h_sb[:, e] if hi_limb else w8lt
                            for kp in range(KP):
                                nc.tensor.matmul(
                                    zps[:],
                                    wt[:, kp, :, gi * 128:(gi + 1) * 128],
                                    u8pair(xt, kp), start=first,
                                    stop=(pi == 2 and kp == KP - 1),
                                    perf_mode=DR)
                                first = False
                        zbt = ptmp.tile([128, R], FP16, tag="tmp", name="zb")
                        nc.scalar.activation(zbt[:], zps[:], AF.Identity)
                        # step-0 gate tanh + Z limb assembly
                        nc.scalar.activation(tg0[:, gi, :], ppg[:, hi, :],
                                             AF.Tanh, bias=bt_sb[:, m:m + 1],
                                             scale=1.0 / LAM)
                        zf = ptmp.tile([128, R], FP16, tag="tmp", name="zf")
                        nc.vector.scalar_tensor_tensor(
                            zf[:], ppg[:, hi, :], bz_sb[:, m:m + 1], zbt[:],
                            op0=ALU.add, op1=ALU.add)
                        nc.gpsimd.tensor_copy(z8_sb[:, e, 0, gi, :], zf[:])
                        nc.vector.tensor_sub(z8_sb[:, e, 1, gi, :], zf[:],
                                             z8_sb[:, e, 0, gi, :])
                # step-0 cell (h=c=r=0): C0 = (t_i+1)*t_g; A2 = (t_o+1)tanh(C0/2)
                nc.gpsimd.scalar_tensor_tensor(cC[:, e, :], tg0[:, 0, :], 1.0,
                                               tg0[:, 2, :], op0=ALU.add,
                                               op1=ALU.mult)
                thc = pth.tile([128, R], FP16, tag="th")
                nc.scalar.activation(thc[:], cC[:, e, :], AF.Tanh, scale=0.5)
                nc.gpsimd.scalar_tensor_tensor(A2[:, e, :], tg0[:, 3, :], 1.0,
                                               thc[:], op0=ALU.add, op1=ALU.mult)
                # u(1) = 0.5*A2 + r0 (r0 = per-episode support mean, exact)
                for ep in range(EPC):
                    cs = slice(ep * 128, (ep + 1) * 128)
                    nc.gpsimd.tensor_scalar(u8a[:, e, cs], A2[:, e, cs], 0.5,
                                            r0_sb[:, e, ep:ep + 1],
                                            op0=ALU.mult, op1=ALU.add)
                    # r1 staging doubles as the q-assembly source: x + r0 for
                    # step-1 logits (overwritten with r(1)+x during step 1)
                    nc.gpsimd.tensor_scalar_add(
                        r1_sb[:, e, cs], xtf_sb[:, e, cs].bitcast(F32),
                        r0_sb[:, e, ep:ep + 1])
            nc.vector.memset(dx8_sb[:, :, :, NK:NKP], 0.0)
            # loads first needed from step 1 on, deferred so the phase-B
            # weight stream owns the queues
            nc.scalar.dma_start(out=i8_sb[:], in_=i8d[:])
            nc.scalar.dma_start(out=i16_sb[:], in_=i16d[:])
            nc.scalar.dma_start(out=s16_sb[:],
                                in_=s16.rearrange("ep c p e -> p ep c e"))
            nc.scalar.dma_start(out=s8d_sb[:],
                                in_=s8d.rearrange("ep (k p) c -> p ep k c", p=128))
            if dbg:
                nc.sync.dma_start(out=dbg["u0"], in_=u8a[:])
                nc.sync.dma_start(out=dbg["a0"], in_=A2[:])

            # ================= steps 1..7 =================
            for t in range(1, n_steps):
                last = (t == n_steps - 1)
                wlo = t in WLO_STEPS
                u8cur, u8nxt = (u8a, u8b) if t % 2 == 1 else (u8b, u8a)

                # ---- attention front: logits/softmax/transposes per ep ----
                if not last:
                    for ep in range(EPC):
                        cs = slice(ep * 128, (ep + 1) * 128)
                        psd = pp1.tile([128, NKP], F32, tag="p1", name="psd")
                        if t <= 2:
                            # exact f32r logits (steps 1-2 are chaos-sensitive)
                            half = EC // 2
                            sview = stf[ep].rearrange("(k p) c -> p k c", p=128)
                            spa = pbig.tile([128, half, NK], F32R, tag="big",
                                            name="spa")
                            spb = pbig.tile([128, half, NK], F32R, tag="big",
                                            name="spb")
                            nc.sync.dma_start(out=spa[:], in_=sview[:, :half, :])
                            nc.scalar.dma_start(out=spb[:], in_=sview[:, half:, :])
                            # q tiles overlay the x8 limb tiles (dead after
                            # phase B): fp8 [128,EC,512] viewed as f32
                            # [128,EC,128], ping-ponged across episodes.
                            qt = x8_sb[:, ep % 2].bitcast(F32)
                            # q(t) = 0.5*A2(t-1) + (x + r(t-1)) with the x+r
                            # part staged fp16 in r1_sb (x+r0 from phase B at
                            # t=1, r(1)+x written during step 1 for t=2)
                            nc.vector.scalar_tensor_tensor(
                                qt[:], A2[:, :, cs], 0.5, r1_sb[:, :, cs],
                                op0=ALU.mult, op1=ALU.add)
                            for k in range(EC):
                                rhs = (spa[:, k, :] if k < half
                                       else spb[:, k - half, :])
                                nc.tensor.matmul(psd[:, :NK],
                                                 qt[:, k, :].bitcast(F32R), rhs,
                                                 start=(k == 0),
                                                 stop=(k == EC - 1))
                            if t == 2:
                                # dots_x = x.S exact -> fp8 limbs for steps 3+
                                dxp = pp1.tile([128, NKP], F32, tag="p1",
                                               name="dxp")
                                for k in range(EC):
                                    rhs = (spa[:, k, :] if k < half
                                           else spb[:, k - half, :])
                                    nc.tensor.matmul(dxp[:, :NK],
                                                     xtf_sb[:, k, cs],
                                                     rhs, start=(k == 0),
                                                     stop=(k == EC - 1))
                                nc.scalar.activation(dx8_sb[:, ep, 0, :NK],
                                                     dxp[:, :NK], AF.Identity)
                                nc.vector.tensor_sub(dx8_sb[:, ep, 1, :NK],
                                                     dxp[:, :NK],
                                                     dx8_sb[:, ep, 0, :NK])
                        else:
                            # fp8 logits: dots_x pair + u8.S8h
                            nc.tensor.matmul(psd[:], i8_sb[:],
                                             dx8_sb[:, ep, :, :], start=True,
                                             stop=False, perf_mode=DR)
                            for kp in range(KP):
                                nc.tensor.matmul(
                                    psd[:], u8pair(u8cur, kp, cs),
                                    s8d_sb[:, ep, 2 * kp:2 * kp + 2, :],
                                    start=False, stop=(kp == KP - 1),
                                    perf_mode=DR)
                        # softmax over NKP (padding exps to ~0 automatically)
                        nw = NK if t <= 2 else NKP
                        nmax = pstat.tile([128, 1], F32, tag="st")
                        nc.vector.tensor_reduce(nmax[:], psd[:, :nw], axis=AX.X,
                                                op=ALU.max, negate=True)
                        at16 = pat.tile([128, NKP], FP16, tag="at16")
                        sm = pstat.tile([128, 1], F32, tag="st")
                        nc.scalar.activation(at16[:, :nw], psd[:, :nw], AF.Exp,
                                             bias=nmax[:, :], accum_out=sm[:, :])
                        rec = pstat.tile([128, 1], F32, tag="st")
                        nc.vector.reciprocal(rec[:], sm[:])
                        nc.gpsimd.tensor_scalar_mul(at16[:, :nw], at16[:, :nw],
                                                    rec[:, :])
                        if t <= 2:
                            # tail cols feed transposes; zero them so the
                            # transposed pad rows stay clean on hardware
                            nc.vector.memset(at16[:, NK:NKP], 0.0)
                        for c in range(NC):
                            pt = pp2.tile([128, 128], FP16, tag="p2", name="pt")
                            nc.tensor.transpose(
                                pt[:], at16[:, c * 128:(c + 1) * 128],
                                i16_sb[:, :])
                            nc.vector.tensor_copy(aT_sb[:, ep, c, :], pt[:])

                # ---- gates + cell update + interleaved r-matmuls ----
                for e in range(EC):
                    zt = z8_sb[:, e]
                    if wlo:
                        w8lt = pw8l.tile([128, KP, 2, 512], FP8, tag="w8l")
                        (nc.scalar if (last or e % 2) else nc.sync).dma_start(
                            out=w8lt[:], in_=w8l[e])
                    tg = pgw.tile([128, 4, R], FP16, tag="gw")
                    for half in range(2):
                        ppg = ppg_pool.tile([128, 2, R], F32, tag="pg")
                        for hi in range(2):
                            gi = 2 * half + hi
                            slot = ppg[:, hi, :]
                            first = True
                            if wlo:
                                for kp in range(KP):
                                    nc.tensor.matmul(
                                        slot,
                                        w8lt[:, kp, :, gi * 128:(gi + 1) * 128],
                                        u8pair(u8cur, kp), start=first,
                                        stop=False, perf_mode=DR)
                                    first = False
                            for kp in range(KP):
                                nc.tensor.matmul(
                                    slot,
                                    w8h_sb[:, e, kp, :, gi * 128:(gi + 1) * 128],
                                    u8pair(u8cur, kp), start=first, stop=False,
                                    perf_mode=DR)
                                first = False
                            nc.tensor.matmul(slot, i8_sb[:], zt[:, :, gi, :],
                                             start=False, stop=True,
                                             perf_mode=DR)
                        nc.scalar.activation(tg[:, 2 * half:2 * half + 2, :],
                                             ppg[:], AF.Tanh, scale=1.0 / LAM)
                    # r-matmuls for this E-chunk: all 4 episodes into one
                    # PSUM bank (fp16 S, one accumulation group)
                    psr4 = None
                    if not last:
                        es = slice(e * 128, (e + 1) * 128)
                        psr4 = pp2.tile([128, EPC, 128], F32, tag="p2",
                                        name="psr4")
                        for ep in range(EPC):
                            for c in range(NC):
                                nc.tensor.matmul(
                                    psr4[:, ep, :], s16_sb[:, ep, c, es],
                                    aT_sb[:, ep, c, :],
                                    start=(ep == 0 and c == 0),
                                    stop=(ep == EPC - 1 and c == NC - 1))
                    # C' = 0.5*(t_f+1)*C + (t_i+1)*t_g   (C = 2c). The last
                    # chunk runs ep-sliced so the next step's dots/gates (which
                    # need the full u8) start ~2.5us earlier.
                    if e < EC - 1:
                        slices = [slice(None)]
                    else:
                        slices = [slice(ep * 128, (ep + 1) * 128)
                                  for ep in range(EPC)]
                    for si, cs2 in enumerate(slices):
                        av = ptmp.tile([128, R], FP16, tag="tmp", name="av")
                        avs = av[:, :R // len(slices)]
                        nc.vector.scalar_tensor_tensor(
                            avs, tg[:, 1, cs2], 1.0, cC[:, e, cs2],
                            op0=ALU.add, op1=ALU.mult)
                        bg = ptmp.tile([128, R], FP16, tag="tmp", name="bg")
                        bgs = bg[:, :R // len(slices)]
                        nc.gpsimd.scalar_tensor_tensor(
                            bgs, tg[:, 0, cs2], 1.0, tg[:, 2, cs2],
                            op0=ALU.add, op1=ALU.mult)
                        nc.vector.scalar_tensor_tensor(
                            cC[:, e, cs2], avs, 0.5, bgs, op0=ALU.mult,
                            op1=ALU.add)
                        thc = pth.tile([128, R], FP16, tag="th")
                        thcs = thc[:, :R // len(slices)]
                        nc.scalar.activation(thcs, cC[:, e, cs2], AF.Tanh,
                                             scale=0.5)
                        eng = nc.vector if len(slices) > 1 else nc.gpsimd
                        eng.scalar_tensor_tensor(A2[:, e, cs2], tg[:, 3, cs2],
                                                 1.0, thcs, op0=ALU.add,
                                                 op1=ALU.mult)
                        if not last:
                            # u(t+1) = 0.5*A2 + r(t)
                            if len(slices) > 1:
                                rsrc = psr4[:, si, :]
                            else:
                                rsrc = psr4.rearrange("p a b -> p (a b)")
                            nc.vector.scalar_tensor_tensor(
                                u8nxt[:, e, cs2], A2[:, e, cs2], 0.5, rsrc,
                                op0=ALU.mult, op1=ALU.add)
                        else:
                            nc.sync.dma_start(out=a2o[:, e, cs2],
                                              in_=A2[:, e, cs2])
                    if t == 1:
                        nc.gpsimd.scalar_tensor_tensor(
                            r1_sb[:, e, :],
                            psr4.rearrange("p a b -> p (a b)"), 1.0,
                            xtf_sb[:, e, :].bitcast(F32),
                            op0=ALU.mult, op1=ALU.add)
                if dbg:
                    nc.sync.dma_start(out=dbg[f"a{t}"], in_=A2[:])
                    if not last:
                        nc.sync.dma_start(out=dbg[f"u{t}"], in_=u8nxt[:])

    nc.compile()
    return nc


def _get_nc(n_steps=KS):
    if n_steps not in _STATE:
        _STATE[n_steps] = _build(n_steps)
    return _STATE[n_steps]


def _chunk_block(w):
    """[128, KP, 2, G] fp8 -> [EC, 128, KP, 2, 512] with chunk e covering
    gate tiles (e, 8+e, 16+e, 24+e)."""
    out = np.empty((EC, 128, KP, 2, 512), dtype=w.dtype)
    for e in range(EC):
        cols = np.concatenate([np.arange(m * 128, (m + 1) * 128)
                               for m in (e, 8 + e, 16 + e, 24 + e)])
        out[e] = w[:, :, :, cols]
    return out


def _prep_in_maps(targets, support_embeddings, W_ih, W_hh, b_ih, b_hh):
    targets = np.asarray(targets, np.float32)
    S_all = np.asarray(support_embeddings, np.float32)
    W_ih = np.asarray(W_ih, np.float32)
    W_hh = np.asarray(W_hh, np.float32)
    b = np.asarray(b_ih, np.float32) + np.asarray(b_hh, np.float32)

    rs = np.full(G, 0.5, np.float32)
    rs[2 * E:3 * E] = 1.0
    Ws_hh = W_hh * rs[:, None] * LAM
    Ws_ih = W_ih * rs[:, None] * LAM

    W8h = Ws_hh.astype(F8NP)
    W8l = (Ws_hh - W8h.astype(np.float32)).astype(F8NP)
    kpair = lambda w: np.ascontiguousarray(
        w.T.reshape(KP, 2, 128, G).transpose(2, 0, 1, 3))   # [128, KP, 2, G]
    w8h_np = _chunk_block(kpair(W8h))
    w8l_np = _chunk_block(kpair(W8l))

    wih_f = np.ascontiguousarray(
        Ws_ih.reshape(GT, 128, EC, 128).transpose(0, 3, 2, 1)).astype(F16NP)
    bias_t = np.ascontiguousarray((b * rs).reshape(GT, 128).T)
    bias_z = np.ascontiguousarray((b * rs * LAM).reshape(GT, 128).T)

    i8_np = np.zeros((128, 2, 128), dtype=F8NP)
    eye = np.eye(128, dtype=np.float32)
    i8_np[:, 0, :] = eye.astype(F8NP)
    i8_np[:, 1, :] = eye.astype(F8NP)
    i16_np = eye.astype(F16NP)

    in_maps = []
    for i in range(CORES):
        x = targets[EPC * i:EPC * (i + 1)].reshape(R, E)
        S = S_all[EPC * i:EPC * (i + 1)].reshape(EPC, NK, E)
        xT = np.ascontiguousarray(x.T)                      # [E, R]
        x8h = xT.astype(F8NP)
        x8l = (xT - x8h.astype(np.float32)).astype(F8NP)

        Spad = np.zeros((EPC, NKP, E), np.float32)
        Spad[:, :NK, :] = S
        stf_np = np.ascontiguousarray(S.transpose(0, 2, 1))  # [EPC, E, NK]
        r0 = S.mean(axis=1)                                  # [EPC, E]
        in_maps.append({
            "wih": wih_f,
            "w8h": w8h_np,
            "w8l": w8l_np,
            "x8": np.stack([x8h, x8l], axis=0),
            "xtf": xT,
            "x16": xT.astype(F16NP),
            "bias_t": bias_t,
            "bias_z": bias_z,
            "stf": stf_np,
            "s8d": np.ascontiguousarray(Spad.transpose(0, 2, 1)).astype(F8NP),
            "s16": np.ascontiguousarray(
                Spad.reshape(EPC, NC, 128, E)).astype(F16NP),
            "r0": np.ascontiguousarray(r0.T.reshape(EC, 128, EPC),
                                       dtype=np.float32),
            "i8": i8_np,
            "i16": i16_np,
        })
    return in_maps


def _finish(a2, x_core):
    """h = 0.5*A2 + x on the host (x exact in f32)."""
    uoT = np.asarray(a2, dtype=np.float32).transpose(1, 0, 2).reshape(E, R) * 0.5
    return (uoT.T + x_core.reshape(R, E)).reshape(EPC, T, E)


def kernel(**inputs):
    nc = _get_nc()
    inputs = {k: np.asarray(v) for k, v in inputs.items()}
    in_maps = _prep_in_maps(**inputs)
    res = run_bass_kernel_spmd(nc, in_maps, core_ids=list(range(CORES)))
    targets = np.asarray(inputs["targets"], np.float32)
    out = np.empty((B, T, E), dtype=np.float32)
    for i in range(CORES):
        out[EPC * i:EPC * (i + 1)] = _finish(res.results[i]["a2o"],
                                             targets[EPC * i:EPC * (i + 1)])
    return out


if __name__ == "__main__":
    nc = _get_nc()
    print("build+compile OK; instructions:",
          sum(len(b.instructions) for f in nc.m.functions for b in f.blocks))
